# revision 51
# baseline (speedup 1.0000x reference)
"""Trainium2 Bass kernel for nn_CELoss_4896262717859 (fp8 DoubleRow, gathered
query columns).

For each query column c = idx_node[k] of a sparse adjacency matrix (diagonal
zeroed), a cross-entropy-style loss over the "lower" (r < c) and "upper"
(r > c) neighbor sets:

    contrib_side(c) = [cnt>0 and poscnt==1] * (log(sum_r m exp(out_r)) - poslogit) / cnt

All per-column quantities are sums  sum_r adj[r,c] * w[r]  for
w in {1, pos, pos*out, exp(out)} -> tensor-engine matvecs with a triangular
(L/U) split, computed ONLY for the distinct idx_node columns (~3218 of 8192),
then combined with multiplicities on the host (O(N+K)).

Sharding: core d handles the distinct query columns falling in column slab
[1024d, 1024(d+1)).  Within a slab, columns are bucketed by the 128-row block
containing their diagonal (the "mixed" block); each of the 8 buckets is padded
to a fixed BCAP=52 slots -> exactly 416 column slots per core, so ONE compiled
program (fixed matmul ranges) serves every core and any input.  Rows are
rotated by 1024d so the mixed blocks always land in local row-tiles 0..7.
Bucket overflow beyond 52 distinct columns (~3% of columns for uniform
idx_node; the padding/offload trade-off was measured on hardware) falls back
to an exact host-side computation for the overflowed columns only.

Non-slab rows that are all-zero across a core's gathered columns (~20% at
density 1/256) are dropped exactly; every non-slab row is uniformly L or U
for all of the core's columns, so gathered tiles mix sides via per-row
weight-half routing.  Everything streams as fp8e4 (adjacency 0/1 exact;
weights hi/mid/lo split -> ~12 mantissa bits): 3.0 MB/core vs the 32 MB
int32 baseline.  Row-tile
pairs run as DoubleRow matmuls (2 fp8 MACs/cell/cycle), halving PE time; the
diagonal tiles pair their shared L/U spans the same way.  The mixed 128-row
block of each column is pre-masked on the host: its lower part (rows < c)
replaces the block in the main slab (covered by the L matmul), its upper part
goes to a small separate diagu[128,416] operand (one extra 52-wide matmul per
diagonal tile).  No on-device casts or mask multiplies remain.

DMA: the adjacency chunks stream through the single gpsimd SWDGE ring in
need order (per-chunk contiguous dram tensors; small leading chunks so
matmuls start early, small final chunk so the compute tail after the last
arrival is short); weights/diagu load concurrently via the sync/scalar HWDGE
rings.  Concurrent chunk rings measured strictly worse (fair-share packet
round-robin delays in-order arrival); per-core streaming tops out ~280 GB/s
regardless of ring mix, making the stream the critical path.  The psum bank
is copied out in halves (ACT+DVE) and written back through both HWDGE rings.
"""

import numpy as np
import ml_dtypes

N = 8192
K = 4096
NCORES = 8
SLAB = N // NCORES        # 1024 columns per slab
P = 128                   # partition / tile edge
NT = N // P               # 64 row tiles
TPC = SLAB // P           # 8 diagonal (mixed) tiles per core
NW = 8                    # weights per side: {1, pos, pl_h, pl_m, pl_l, e_h, e_m, e_l}
M = 2 * NW                # 16 psum partitions (L half = 0:8, U half = 8:16)
VW = 16                   # weight-variant stride (cols); == M, and 16B for fp8
CAP = 416                 # column slots per core (one psum bank)
BCAP = CAP // TPC         # 52 slots per 128-row bucket
def _chunk_plan(nt2):
    """DMA chunk sizes summing to nt2: small head (diag tiles first), even
    mid chunks, small tail (so little compute remains after the last DMA
    completion)."""
    sizes = [2, 2, 4]
    rem = nt2 - 8 - 4        # reserve a 4-tile tail chunk
    while rem > 0:
        take = min(12, rem)
        if rem - take == 2:
            take -= 2
        sizes.append(take)
        rem -= take
    sizes.append(4 + rem)    # rem is 0 or negative correction for tiny nt2
    return tuple(sizes)

BF16 = ml_dtypes.bfloat16
FP8 = ml_dtypes.float8_e4m3fn

_BASS_CACHE = {}


def _build_bass(nt2):
    import concourse.tile as tile
    import concourse.mybir as mybir
    from concourse import bacc

    chunk_tiles = _chunk_plan(nt2)
    f8 = mybir.dt.float8e4
    DR = mybir.MatmulPerfMode.DoubleRow
    # Bacc (not raw Bass): its compile() runs generate_event_semaphores,
    # which splits multi-sem waits — TRN2 instructions hold at most one.
    nc = bacc.Bacc("TRN2")
    adjs = [
        nc.dram_tensor(f"adj{ci}", [P, nt * CAP], f8, kind="ExternalInput")
        for ci, nt in enumerate(chunk_tiles)
    ]
    wmat = nc.dram_tensor("wmat", [P, (nt2 + TPC) * VW], f8, kind="ExternalInput")
    diagu = nc.dram_tensor("diagu", [P, CAP], f8, kind="ExternalInput")
    stats = nc.dram_tensor("stats", [M, CAP], mybir.dt.float32, kind="ExternalOutput")

    with tile.TileContext(nc) as tc:
        with (
            tc.tile_pool(name="singles", bufs=1) as singles,
            tc.tile_pool(name="psum", bufs=1, space="PSUM") as psum_pool,
        ):
            # weights/diagu on the two HWDGE rings (needed first; they drain
            # while the chunk stream starts), adjacency chunks on the gpsimd
            # SWDGE ring as one in-order FIFO.
            asb = singles.tile([P, nt2 + TPC, VW], f8)
            nc.sync.dma_start(out=asb, in_=wmat[:, :])
            dsb = singles.tile([P, CAP], f8)
            nc.scalar.dma_start(out=dsb, in_=diagu[:, :])
            chunks = []   # (tile, first_tile, ntiles)
            t0 = 0
            for ci, nt in enumerate(chunk_tiles):
                t = singles.tile([P, nt, CAP], f8, name=f"ch{ci}")
                nc.gpsimd.dma_start(out=t, in_=adjs[ci][:, :])
                chunks.append((t, t0, nt))
                t0 += nt
            assert t0 == nt2

            acc = psum_pool.tile([M, CAP], mybir.dt.float32, name="acc")

            def wv(v, n=1):
                return asb[:, v : v + n, :]

            def du(b):  # diagu strip for mixed block b
                return dsb[:, BCAP * b : BCAP * (b + 1)]

            def chunk_rhs(j, n=1):
                for t, t0, nt in chunks:
                    if t0 <= j and j + n <= t0 + nt:
                        return t[:, j - t0 : j - t0 + n, :]
                raise AssertionError(f"tile {j}+{n} spans chunks")

            def mm(out_ap, w, rhs, **kw):
                nc.tensor.matmul(out_ap, w, rhs, skip_group_check=True,
                                 start=kw.pop("start", False),
                                 stop=kw.pop("stop", False), **kw)

            # Diagonal (mixed) tiles, DoubleRow-paired on their shared spans.
            # Tile j's L span is [64j, 512) (its own mixed bucket holds
            # host-pre-masked lower data), U span is [0, 64j); the mixed
            # bucket's upper part comes from the diagu strips.
            for j in range(0, TPC, 2):
                a, b = BCAP * j, BCAP * (j + 1)
                rhs2 = chunk_rhs(j, 2)
                # shared L span of the pair
                mm(acc[:, b:CAP], wv(j, 2), rhs2[:, :, b:CAP],
                   start=(j == 0), perf_mode=DR)
                # tile j's extra L strip (its own bucket)
                mm(acc[:, a:b], wv(j), chunk_rhs(j)[:, :, a:b])
                if j > 0:
                    # shared U span of the pair
                    mm(acc[:, 0:a], wv(nt2 + j, 2), rhs2[:, :, 0:a], perf_mode=DR)
                # tile j+1's extra U strip (tile j's bucket columns)
                mm(acc[:, a:b], wv(nt2 + j + 1), chunk_rhs(j + 1)[:, :, a:b])
                # upper parts of the mixed buckets themselves
                mm(acc[:, a:b], wv(nt2 + j), du(j))
                mm(acc[:, b : b + BCAP], wv(nt2 + j + 1), du(j + 1))
            for j in range(TPC, nt2, 2):
                mm(acc[:, :], wv(j, 2), chunk_rhs(j, 2),
                   stop=(j == nt2 - 2), perf_mode=DR)

            out_sb = singles.tile([M, CAP], mybir.dt.float32)
            half = CAP // 2
            nc.vector.tensor_copy(out_sb[:, half:], acc[:, half:])
            nc.scalar.copy(out_sb[:, 0:half], acc[:, 0:half])
            nc.sync.dma_start(out=stats[:, 0:half], in_=out_sb[:, 0:half])
            nc.scalar.dma_start(out=stats[:, half:], in_=out_sb[:, half:])

    nc.compile()
    return nc


def _split_fp8(v, terms=3):
    """Split f64 vector into `terms` fp8 values summing to ~v (12 mantissa bits)."""
    out = []
    r = np.asarray(v, np.float64)
    for _ in range(terms):
        t = r.astype(FP8)
        out.append(t)
        r = r - t.astype(np.float64)
    return out


def _make_wside(outputs, targets):
    """Per-row weight table [N, 8] fp8."""
    out = np.asarray(outputs, np.float64).reshape(-1)
    pos = (np.asarray(targets).reshape(-1) != 0).astype(np.float64)
    cols = [np.ones(N, FP8), pos.astype(FP8)]
    cols += _split_fp8(pos * out)
    cols += _split_fp8(np.exp(out))
    return np.stack(cols, axis=1).astype(FP8)  # [N, 8]


def _prepare(outputs, targets, node_adj, idx_node):
    """Build per-core in_maps + combine context (slot->column map, multiplicities,
    host-computed contribution of any bucket-overflow columns).

    Non-slab rows that are all-zero across the core's gathered columns are
    dropped (exact: a zero row contributes nothing to any stat) -> ~20% fewer
    streamed rows at density 1/256.  Every non-slab row is uniformly L or U
    for all of the core's columns, so gathered tiles may mix sides: each row's
    weights go in the L or U half of its tile's weight variant."""
    node_adj = np.asarray(node_adj)
    idx = np.asarray(idx_node).reshape(-1).astype(np.int64)
    ucols, mult = np.unique(idx, return_counts=True)
    wside = _make_wside(outputs, targets)

    slot_cols = np.full((NCORES, CAP), -1, np.int64)
    overflow = []
    rows128 = np.arange(P)
    s_idx = np.arange(CAP)
    base = P * (s_idx // BCAP)  # first local row of each slot's mixed block

    per_core = []
    for d in range(NCORES):
        lo = SLAB * d
        uc = ucols[(ucols >= lo) & (ucols < lo + SLAB)]
        cols_s = np.full(CAP, -1, np.int64)
        for b in range(TPC):
            blk = uc[(uc - lo) // P == b]
            if len(blk) > BCAP:
                overflow.extend(blk[BCAP:].tolist())
                blk = blk[:BCAP]
            cols_s[BCAP * b : BCAP * b + len(blk)] = blk
        slot_cols[d] = cols_s
        valid = cols_s >= 0

        G = (node_adj[:, np.where(valid, cols_s, 0)] != 0).astype(np.float32)
        G[:, ~valid] = 0.0
        # rotate rows: local row r = absolute row (r + 1024d) mod N
        G = np.concatenate([G[lo:], G[:lo]], axis=0)
        lc = np.where(valid, cols_s - lo, -1)  # local split row (diag) per slot
        G[lc[valid], s_idx[valid]] = 0.0       # zero the diagonal
        block = G[base[None, :] + rows128[:, None], s_idx[None, :]]  # [128, CAP]
        lrow = base[None, :] + rows128[:, None]
        diagL = np.where(lrow < lc[None, :], block, 0.0)
        diagU = np.where(lrow > lc[None, :], block, 0.0)
        G[base[None, :] + rows128[:, None], s_idx[None, :]] = diagL
        keep = np.nonzero(G[SLAB:].any(axis=1))[0] + SLAB  # nonzero non-slab rows
        per_core.append((G, keep, diagU))

    ntg = max((len(k) + P - 1) // P for _, k, _ in per_core)
    ntg = max(2, ntg + (ntg % 2))       # even, >=1 pair
    nt2 = TPC + ntg
    chunk_tiles = _chunk_plan(nt2)

    in_maps = []
    for d, (G, keep, diagU) in enumerate(per_core):
        lo = SLAB * d
        rows = np.zeros((nt2 * P, CAP), np.float32)
        rows[0:SLAB] = G[0:SLAB]
        rows[SLAB : SLAB + len(keep)] = G[keep]
        # weight variants: tiles 0..7 diag (L; U twin at nt2+j), 8.. gathered
        w = np.zeros((P, nt2 + TPC, VW), dtype=FP8)
        for j in range(TPC):
            t = (TPC * d + j) % NT
            wr = wside[t * P : (t + 1) * P, :]
            w[:, j, 0:NW] = wr
            w[:, nt2 + j, NW:M] = wr
        absr = (keep + lo) % N
        is_u = keep < (N - lo)          # non-wrapped -> rows above slab -> U
        rw = wside[absr]
        for j in range(TPC, nt2):
            g0 = (j - TPC) * P
            sel = slice(g0, min(g0 + P, len(keep)))
            k = sel.stop - sel.start
            if k <= 0:
                break
            pos = np.arange(k)
            tw = np.zeros((P, VW), dtype=FP8)
            u = is_u[sel]
            tw[pos[u], NW:M] = rw[sel][u]
            tw[pos[~u], 0:NW] = rw[sel][~u]
            w[:, j, :] = tw
        adjf = (
            rows.reshape(nt2, P, CAP).transpose(1, 0, 2).reshape(P, nt2 * CAP)
        ).astype(FP8)
        im = {
            "wmat": np.ascontiguousarray(w.reshape(P, (nt2 + TPC) * VW)),
            "diagu": np.ascontiguousarray(diagU.astype(FP8)),
        }
        t0 = 0
        for ci, nt in enumerate(chunk_tiles):
            im[f"adj{ci}"] = np.ascontiguousarray(
                adjf[:, t0 * CAP : (t0 + nt) * CAP]
            )
            t0 += nt
        in_maps.append(im)

    mult_of = np.zeros(N, np.int64)
    mult_of[ucols] = mult
    over_loss = _host_cols_loss(outputs, targets, node_adj, overflow, mult_of)
    ctx = {
        "slot_cols": slot_cols,
        "mult_of": mult_of,
        "over_loss": over_loss,
        "nt2": nt2,
    }
    return in_maps, ctx


def _host_cols_loss(outputs, targets, node_adj, cols, mult_of):
    """Reference-exact loss contribution of a few columns (bucket overflow only)."""
    if not cols:
        return 0.0
    cols = np.asarray(cols, np.int64)
    out = np.asarray(outputs, np.float64).reshape(-1)
    pos = np.asarray(targets).reshape(-1) != 0
    A = node_adj[:, cols] != 0
    r = np.arange(N)[:, None]
    A = A & (r != cols[None, :])
    total = 0.0
    for mask in (A & (r < cols[None, :]), A & (r > cols[None, :])):
        cnt = mask.sum(axis=0)
        poscnt = (mask & pos[:, None]).sum(axis=0)
        sumexp = (mask * np.exp(out)[:, None]).sum(axis=0)
        poslogit = (mask * (pos * out)[:, None]).sum(axis=0)
        valid = (cnt > 0) & (poscnt == 1)
        contrib = np.where(
            valid,
            (np.log(np.maximum(sumexp, 1e-300)) - poslogit) / np.maximum(cnt, 1),
            0.0,
        )
        total += (contrib * mult_of[cols]).sum()
    return total


def _combine(stats_list, ctx):
    """Per-core stats [16, CAP] f32 -> scalar loss (f64 math)."""

    def side_contrib(x):
        cnt, poscnt = x[0], x[1]
        poslogit = x[2] + x[3] + x[4]
        sumexp = x[5] + x[6] + x[7]
        valid = (cnt > 0.5) & (np.abs(poscnt - 1.0) < 0.25)
        lse = np.log(np.where(valid, np.maximum(sumexp, 1e-300), 1.0))
        return np.where(valid, (lse - poslogit) / np.maximum(cnt, 1.0), 0.0)

    total = ctx["over_loss"]
    for d, s in enumerate(stats_list):
        x = np.asarray(s, np.float64)
        contrib = side_contrib(x[0:NW]) + side_contrib(x[NW:M])
        cols = ctx["slot_cols"][d]
        valid = cols >= 0
        total += (contrib[valid] * ctx["mult_of"][cols[valid]]).sum()
    return np.array(total, dtype=np.float32)


def _ensure_axon_hooks_stub():
    """bass_utils imports antenv.axon_hooks when tracing is requested via
    env; the module is absent on some images. Provide a no-op stub so the
    import never crashes (hook=None -> bass_utils skips tracing)."""
    import sys
    import types

    try:
        import antenv.axon_hooks  # noqa: F401
    except ImportError:
        mod = types.ModuleType("antenv.axon_hooks")
        state = {"hook": None}
        mod.set_axon_ntff_profile_hook = lambda h: state.__setitem__("hook", h)
        mod.get_axon_ntff_profile_hook = lambda: state["hook"]
        sys.modules["antenv.axon_hooks"] = mod


def _device_stats(in_maps, nt2):
    _ensure_axon_hooks_stub()
    from concourse.bass_utils import run_bass_kernel_spmd

    if nt2 not in _BASS_CACHE:
        _BASS_CACHE[nt2] = _build_bass(nt2)
    last_exc = None
    for attempt in range(4):
        try:
            res = run_bass_kernel_spmd(
                _BASS_CACHE[nt2], in_maps, core_ids=list(range(NCORES))
            )
            return [r["stats"] for r in res.results]
        except Exception as e:  # transient NRT/accelerator hiccups
            last_exc = e
            try:
                # a fresh PJRT client usually recovers a transiently
                # "unrecoverable" accelerator; mirrors a process restart
                import jax
                import jax.extend.backend as _jeb

                jax.clear_caches()
                _jeb.clear_backends()
            except Exception:
                pass
            import time

            time.sleep(2.0 * (attempt + 1))
    raise last_exc


def _sim_stats(in_maps, nt2):
    """Numpy emulation of the device kernel (same inputs), for logic validation."""
    outs = []
    nch = len(_chunk_plan(nt2))
    for m in in_maps:
        adjf = np.concatenate(
            [m[f"adj{ci}"] for ci in range(nch)], axis=1
        ).astype(np.float32)
        diagu = m["diagu"].astype(np.float32)
        w = m["wmat"].reshape(P, nt2 + TPC, VW).astype(np.float32)
        acc = np.zeros((M, CAP), np.float32)
        for j in range(nt2):
            tile = adjf[:, j * CAP : (j + 1) * CAP]
            if j < TPC:
                c0 = BCAP * j
                acc[:, c0:] += w[:, j, :M].T @ tile[:, c0:]
                acc[:, :c0] += w[:, nt2 + j, :M].T @ tile[:, :c0]
                acc[:, c0 : c0 + BCAP] += w[:, nt2 + j, :M].T @ diagu[:, c0 : c0 + BCAP]
            else:
                acc += w[:, j, :M].T @ tile
        outs.append(acc)
    return outs


def kernel(outputs, targets, node_adj, idx_node, _simulate=False):
    in_maps, ctx = _prepare(outputs, targets, node_adj, idx_node)
    nt2 = ctx["nt2"]
    stats = _sim_stats(in_maps, nt2) if _simulate else _device_stats(in_maps, nt2)
    return _combine(stats, ctx)
